# revision 14
# baseline (speedup 1.0000x reference)
"""Batch-softmax attention for Trainium2 (8 NeuronCores) — diff-softmax.

Problem: out[b,h,i,v] = sum_j softmax_over_b(QK^T/sqrt(H))[b,h,i,j] * V[b,h,j,v]
with B=4, H=8, S=2048, D=64.  Softmax over the BATCH axis (dim=0).
Sharding: one head per NeuronCore (H=8), so the batch softmax is core-local.

Restructure vs the previous 170.7us kernel (which was ScalarE+VectorE
co-bound: exp ACTIVATE 150.8us busy, DVE 163us busy):

  softmax_b(s_0..s_3) = [r, e1*r, e2*r, e3*r],  e_b = exp(s_b - s_0),
                        r = 1/(1 + e1 + e2 + e3)

  - PE computes the DIFFS d_b = (q_b.k_b - q_0.k_0)/1 directly as ONE K=128
    matmul per b in {1,2,3}: lhsT = [k_b^T; k_0^T] (64+64 rows stacked),
    rhs = [q_b^T; -q_0^T].  3 score streams instead of 4.
  - ScalarE: ONE exp per j-tile over [128, 3*512] PSUM -> E bf16
    (4 -> 3 exp units: ACTIVATE 150.8 -> 102.5us busy).
  - VectorE: SA = e1+e2 (TT 2x), R = recip_1NR(1 + SA + e3) (custom DVE op
    with the +1 folded in; R = w0 directly, so b=0 needs no multiply),
    W123 = E * bcast(R) (TT 2x).  DVE 163 -> 118us busy.
  - WV matmuls: V bf16 stationary, rhs = R / W123 planes, col-tiled pairs
    (M=64 at (0,0)/(0,64)) accumulating out_T[v,i] in one PSUM tile.

PSUM: 2 rolling score tiles x 3 banks + 1 accumulator tile x 2 banks = 8.
Measured on trn2 (8 cores): ~150.1us NEFF exec, rel L2 err 2.5e-3
(baseline it replaces: 171.2us / 2.1e-3).

Engine accounting at 150us: VectorE 118us busy (92% packed in its window)
is the pacer; ScalarE 102.5us; PE ~75% busy mostly HAM-warm; ~14us
preamble+ramp, ~8us drain/postamble.  Rejected variants (measured): SA on
GPSIMD 198us (Pool sem/dispatch overhead + SBUF-port contention degrades
concurrent DVE 2x ops to 1x); e1+e2 via PE identity-matmul accumulation
184us (PE FIFO head-of-line blocking on the exp->idMM dependency).
"""

import math
import os
import sys

import numpy as np

sys.path.insert(0, "/opt/trn_rl_repo")
os.environ.setdefault("MYCRO_LOCAL_CACHE", "1")

B, H, S, D = 4, 8, 2048, 64
N_CORES = 8
SCALE = 1.0 / math.sqrt(H)  # NOTE: reference scales by sqrt(num_heads)

IC = 4          # i-chunks of 512 columns
ICW = S // IC   # 512
JT = S // 128   # 16 j-tiles of 128 rows
MID_JG = int(os.environ.get("K_MID_JG", "2"))
PSP_BUFS = int(os.environ.get("K_PSP_BUFS", "2"))
GRP_BUFS = int(os.environ.get("K_GRP_BUFS", "8"))
HEADJ = int(os.environ.get("K_HEADJ", "4"))  # starter j-tiles (0 = off)
POOL_MODE = os.environ.get("K_POOL_MODE", "stack")

_CACHED_NC = None
_DVE_OPS = {}


def _register_dve_op(name, body_fn, ref):
    """Register a custom DVE op once; returns the DveOp."""
    if name in _DVE_OPS:
        return _DVE_OPS[name]
    import concourse.dve_ops as dvo
    from concourse.dve_spec import Spec, lower
    from concourse.dve_uop import DveOpSpec

    existing = [o for o in dvo.OPS if o.name == name]
    if existing:
        _DVE_OPS[name] = existing[0]
        return existing[0]

    op = dvo.DveOp(name, Spec(body=body_fn(), reference=ref), subdim=False,
                   uops_sha={})
    dvo.OPS.append(op)
    dvo.CUSTOM_DVE_SPECS[name] = op.spec
    dvo._SUB_OPCODE_FOR_NAME[name] = dvo._CUSTOM_DVE_ROW_BASE + len(dvo.OPS) - 1
    assert dvo._SUB_OPCODE_FOR_NAME[name] < 0x20
    shas = {}
    for ver in ("v3", "v4"):
        s = DveOpSpec(name=name, opcode=dvo.get_dve_sub_opcode(name),
                      uops=lower(op.spec, ver=ver), rd1_en=True)
        shas[ver] = s.sha(ver)
    object.__setattr__(op, "uops_sha", shas)
    _DVE_OPS[name] = op
    return op


def _register_add1p_recip():
    """Custom DVE op: out = recip_approx(in0 + in1 + 1), 1 Newton step."""
    import numpy as np_
    from concourse.dve_spec import AluOp, Bin, C0, C1, One, Src0, Src1

    def _body():
        _x = (Src0 + Src1) + One
        _nx = Bin(AluOp.BITWISE_NOT, _x, _x)
        _y0 = _nx * C0
        return _y0 * (C1 - _x * _y0)

    def _ref(in0, in1, s0, s1, imm2):
        x = (in0 + in1 + 1.0).astype(np_.float32)
        nx = (~x.view(np_.int32)).view(np_.float32)
        y0 = nx * np_.float32(s0)
        return y0 * (np_.float32(s1) - x * y0)

    return _register_dve_op("ADD1P_RECIP_1NR_ANT", _body, _ref)


def _build_nc():
    from concourse import bacc, tile
    from concourse.bass import mybir
    from concourse.dve_ops import RECIP_APPROX_FAST_CONSTS

    add1p_recip = _register_add1p_recip()

    f32 = mybir.dt.float32
    f16 = mybir.dt.float16
    bf16 = mybir.dt.bfloat16
    Exp = mybir.ActivationFunctionType.Exp
    rc = RECIP_APPROX_FAST_CONSTS

    nc = bacc.Bacc("TRN2", target_bir_lowering=False, debug=False)

    # kq[bi] = [kst_b | qst_b] for b = bi+1: kst rows 0:64 = k_b^T,
    # 64:128 = k_0^T; qst rows 0:64 = q_b^T, 64:128 = -q_0^T.
    kq_in = nc.dram_tensor("kq", [3, 2, 128, S], f16, kind="ExternalInput").ap()
    v_in = nc.dram_tensor("v", [B, 128, JT, D], bf16, kind="ExternalInput").ap()
    out_d = nc.dram_tensor("out", [B, D, S], f32, kind="ExternalOutput").ap()
    # starter blob: kst/qst slices for the first HEADJ j-tiles of chunk 0 in
    # one small leading DMA so the pipeline ramps before the full kq loads:
    # [kst1h | kst2h | kst3h | qst1c0 | qst2c0 | qst3c0]
    W_HEAD = 3 * HEADJ * 128 + 3 * ICW
    qkh_in = (nc.dram_tensor("qkh", [128, W_HEAD], f16,
                             kind="ExternalInput").ap() if HEADJ else None)

    with tile.TileContext(nc, pool_alloc_mode=POOL_MODE) as tc:
        with (
            tc.tile_pool(name="wts", bufs=1) as wpool,
            tc.tile_pool(name="grp", bufs=GRP_BUFS) as gpool,
            tc.tile_pool(name="osb", bufs=2) as opool,
            tc.tile_pool(name="ps", bufs=PSP_BUFS, space="PSUM") as psp,
            tc.tile_pool(name="po", bufs=1, space="PSUM") as pop,
        ):
            KQ = [wpool.tile([128, 2, S], f16, tag=f"kq{b}", name=f"KQ{b}")
                  for b in range(3)]
            V = wpool.tile([128, B, JT, D], bf16, tag="v")
            if HEADJ:
                ST = wpool.tile([128, W_HEAD], f16, tag="qkh")
                nc.sync.dma_start(out=ST[:], in_=qkh_in)
            for b in range(3):
                nc.sync.dma_start(out=KQ[b][:],
                                  in_=kq_in[b].transpose([1, 0, 2]))
            nc.sync.dma_start(out=V[:], in_=v_in.transpose([1, 0, 2, 3]))

            def schedule(ic):
                # taper group size down at every chunk boundary: big WV
                # bunches queued at a boundary block the next chunk's diff
                # matmuls in the PE FIFO and stall VectorE 2-4us per chunk
                if MID_JG == 2:
                    if ic == 0:
                        return [1, 1] + [2] * 7
                    if ic == IC - 1:
                        return [2] * 7 + [1, 1]
                    return [2] * 8
                if ic == 0:
                    return [1, 1, 2, 4, 4, 2, 2]
                if ic == IC - 1:
                    return [2, 4, 4, 2, 2, 1, 1]
                return [2, 4, 4, 4, 2]

            def emit_group(ic, JG, j0, out_t):
                isl = slice(ic * ICW, (ic + 1) * ICW)
                E4 = gpool.tile([128, JG, 3, ICW], bf16, tag="E4")
                for j4 in range(JG):
                    j = j0 + j4
                    sp = psp.tile([128, 3, ICW], f32, tag="sp")
                    for b in range(3):
                        if HEADJ and ic == 0 and j < HEADJ:
                            kt = ST[:, (b * HEADJ + j) * 128:
                                    (b * HEADJ + j + 1) * 128]
                            qt = ST[:, 3 * HEADJ * 128 + b * ICW:
                                    3 * HEADJ * 128 + (b + 1) * ICW]
                        else:
                            kt = KQ[b][:, 0, j * 128:(j + 1) * 128]
                            qt = KQ[b][:, 1, isl]
                        # d_b_T[j, i] = k_b.q_b - k_0.q_0 (K=128 stacked)
                        nc.tensor.matmul(sp[:, b, :], kt, qt,
                                         start=True, stop=True)
                    nc.scalar.activation(E4[:, j4, :, :], sp[:], Exp,
                                         scale=SCALE)

                # SA = e1 + e2; R = 1 / (1 + SA + e3) = w0.  Pair-granular so
                # VectorE can start as soon as two exps land, not the whole
                # group.
                SA = gpool.tile([128, JG, ICW], bf16, tag="SA")
                R = gpool.tile([128, JG, ICW], bf16, tag="R")
                for p in range(0, JG, 2):
                    pe = min(p + 2, JG)
                    nc.vector.tensor_add(SA[:, p:pe, :], E4[:, p:pe, 0, :],
                                         E4[:, p:pe, 1, :])
                    nc.vector._custom_dve(
                        add1p_recip, out=R[:, p:pe, :], in0=SA[:, p:pe, :],
                        in1=E4[:, p:pe, 2, :], s0=rc["s0"], s1=rc["s1"])
                # W[p, j4, b-1, i] = e_b * r for b in 1..3
                W = gpool.tile([128, JG, 3, ICW], bf16, tag="W")
                rb = R[:].unsqueeze(2).broadcast_to([128, JG, 3, ICW])
                nc.vector.tensor_mul(W[:], E4[:], rb)

                # out_T[v, i] += col-tiled accumulation over j
                for j4 in range(JG):
                    j = j0 + j4
                    rhs = [R[:, j4, :], W[:, j4, 0, :],
                           W[:, j4, 1, :], W[:, j4, 2, :]]
                    for b, (pair, base) in enumerate(
                        [(0, 0), (0, 64), (1, 0), (1, 64)]
                    ):
                        nc.tensor.matmul(
                            out_t[base:base + 64, pair, :],
                            V[:, b, j, :], rhs[b],
                            start=(j == 0), stop=(j == JT - 1),
                            tile_position=(0, base), skip_group_check=True)

            def emit_outputs(ic, out_t):
                isl = slice(ic * ICW, (ic + 1) * ICW)
                OSB = opool.tile([128, 2, ICW], f32, tag="osb")
                last = ic == IC - 1
                eng23 = nc.scalar if last else nc.sync
                if last:
                    # VectorE is idle in the drain chunk: split the copy
                    nc.scalar.copy(OSB[:, 0, :], out_t[:, 0, :])
                    nc.vector.tensor_copy(OSB[:, 1, :], out_t[:, 1, :])
                else:
                    nc.scalar.copy(OSB[:], out_t[:])
                nc.sync.dma_start(out=out_d[0, :, isl], in_=OSB[0:64, 0, :])
                nc.sync.dma_start(out=out_d[1, :, isl], in_=OSB[64:128, 0, :])
                eng23.dma_start(out=out_d[2, :, isl], in_=OSB[0:64, 1, :])
                eng23.dma_start(out=out_d[3, :, isl], in_=OSB[64:128, 1, :])

            for ic in range(IC):
                out_t = pop.tile([128, 2, ICW], f32, tag="oacc")
                j0 = 0
                for JG in schedule(ic):
                    emit_group(ic, JG, j0, out_t)
                    j0 += JG
                emit_outputs(ic, out_t)

    nc.compile()
    return nc


def _get_nc():
    global _CACHED_NC
    if _CACHED_NC is None:
        _CACHED_NC = _build_nc()
    return _CACHED_NC


def _make_in_maps(query, key, value):
    import ml_dtypes
    q16 = query.astype(np.float16)
    k16 = key.astype(np.float16)
    vbf = value.astype(ml_dtypes.bfloat16)
    in_maps = []
    for h in range(H):
        qt = q16[:, h].transpose(0, 2, 1)  # [B, D, S]
        kt = k16[:, h].transpose(0, 2, 1)
        nq0 = -qt[0]
        kq = np.empty((3, 2, 128, S), np.float16)
        for bi in range(3):
            kq[bi, 0, 0:64] = kt[bi + 1]
            kq[bi, 0, 64:128] = kt[0]
            kq[bi, 1, 0:64] = qt[bi + 1]
            kq[bi, 1, 64:128] = nq0
        vv = vbf[:, h].reshape(B, JT, 128, D).transpose(0, 2, 1, 3)
        im = {
            "kq": np.ascontiguousarray(kq),
            "v": np.ascontiguousarray(vv),
        }
        if HEADJ:
            im["qkh"] = np.ascontiguousarray(np.concatenate(
                [kq[bi, 0, :, 0:HEADJ * 128] for bi in range(3)]
                + [kq[bi, 1, :, 0:ICW] for bi in range(3)], axis=1))
        in_maps.append(im)
    return in_maps


def _assemble(results):
    out = np.empty((B, H, S, D), np.float32)
    for h in range(H):
        out[:, h] = results[h]["out"].transpose(0, 2, 1)  # [B,D,S] -> [B,S,D]
    return out


def _install_profile_hook():
    """Provide antenv.axon_hooks with a ctypes NTFF profile hook."""
    import contextlib
    import ctypes
    import types

    try:
        from antenv.axon_hooks import get_axon_ntff_profile_hook  # noqa: F401
        return
    except ImportError:
        pass

    so_path = "/opt/axon/libaxon_pjrt.so"
    lib = ctypes.CDLL(so_path)
    if not hasattr(lib, "axon_start_nrt_profile"):
        return
    lib.axon_start_nrt_profile.argtypes = [
        ctypes.POINTER(ctypes.c_int64), ctypes.c_size_t]
    lib.axon_start_nrt_profile.restype = ctypes.c_int64
    lib.axon_stop_nrt_profile.argtypes = [ctypes.c_char_p]
    lib.axon_stop_nrt_profile.restype = ctypes.c_int64

    @contextlib.contextmanager
    def _hook(output_dir, device_ids):
        import jax
        jax.devices()
        if device_ids:
            ids = (ctypes.c_int64 * len(device_ids))(*device_ids)
            rc = lib.axon_start_nrt_profile(ids, len(device_ids))
        else:
            rc = lib.axon_start_nrt_profile(None, 0)
        if rc != 0:
            raise RuntimeError(f"axon_start_nrt_profile rc={rc}")
        try:
            yield
        finally:
            n = lib.axon_stop_nrt_profile(str(output_dir).encode())
            print(f"ntff profile: {n} file(s) written to {output_dir}")

    mod = types.ModuleType("antenv.axon_hooks")
    mod.get_axon_ntff_profile_hook = lambda: _hook
    mod.set_axon_ntff_profile_hook = lambda h: None
    sys.modules["antenv.axon_hooks"] = mod


def run(query, key, value, trace=False):
    """Run the distributed kernel; returns (output, exec_time_ns or None)."""
    from concourse.bass_utils import run_bass_kernel_spmd

    if trace:
        _install_profile_hook()
    nc = _get_nc()
    in_maps = _make_in_maps(query, key, value)
    res = run_bass_kernel_spmd(nc, in_maps, core_ids=list(range(N_CORES)),
                               trace=trace)
    return _assemble(res.results), res.exec_time_ns


def kernel(query, key, value):
    out, _ = run(query, key, value, trace=False)
    return out


# revision 15
# speedup vs baseline: 1.0100x; 1.0100x over previous
"""Batch-softmax attention for Trainium2 (8 NeuronCores) — diff-softmax.

Problem: out[b,h,i,v] = sum_j softmax_over_b(QK^T/sqrt(H))[b,h,i,j] * V[b,h,j,v]
with B=4, H=8, S=2048, D=64.  Softmax over the BATCH axis (dim=0).
Sharding: one head per NeuronCore (H=8), so the batch softmax is core-local.

Restructure vs the previous 170.7us kernel (which was ScalarE+VectorE
co-bound: exp ACTIVATE 150.8us busy, DVE 163us busy):

  softmax_b(s_0..s_3) = [r, e1*r, e2*r, e3*r],  e_b = exp(s_b - s_0),
                        r = 1/(1 + e1 + e2 + e3)

  - PE computes the DIFFS d_b = (q_b.k_b - q_0.k_0)/1 directly as ONE K=128
    matmul per b in {1,2,3}: lhsT = [k_b^T; k_0^T] (64+64 rows stacked),
    rhs = [q_b^T; -q_0^T].  3 score streams instead of 4.
  - ScalarE: ONE exp per j-tile over [128, 3*512] PSUM -> E bf16
    (4 -> 3 exp units: ACTIVATE 150.8 -> 102.5us busy).
  - VectorE: SA = e1+e2 (TT 2x), R = recip_1NR(1 + SA + e3) (custom DVE op
    with the +1 folded in; R = w0 directly, so b=0 needs no multiply),
    W123 = E * bcast(R) (TT 2x).  DVE 163 -> 118us busy.
  - WV matmuls: V bf16 stationary, rhs = R / W123 planes, col-tiled pairs
    (M=64 at (0,0)/(0,64)) accumulating out_T[v,i] in one PSUM tile.

PSUM: 2 rolling score tiles x 3 banks + 1 accumulator tile x 2 banks = 8.
Measured on trn2 (8 cores): ~150.1us NEFF exec, rel L2 err 2.5e-3
(baseline it replaces: 171.2us / 2.1e-3).

Engine accounting at 150us: VectorE 118us busy (92% packed in its window)
is the pacer; ScalarE 102.5us; PE ~75% busy mostly HAM-warm; ~14us
preamble+ramp, ~8us drain/postamble.  Rejected variants (measured): SA on
GPSIMD 198us (Pool sem/dispatch overhead + SBUF-port contention degrades
concurrent DVE 2x ops to 1x); e1+e2 via PE identity-matmul accumulation
184us (PE FIFO head-of-line blocking on the exp->idMM dependency).
"""

import math
import os
import sys

import numpy as np

sys.path.insert(0, "/opt/trn_rl_repo")
os.environ.setdefault("MYCRO_LOCAL_CACHE", "1")

B, H, S, D = 4, 8, 2048, 64
N_CORES = 8
SCALE = 1.0 / math.sqrt(H)  # NOTE: reference scales by sqrt(num_heads)

IC = 4          # i-chunks of 512 columns
ICW = S // IC   # 512
JT = S // 128   # 16 j-tiles of 128 rows
MID_JG = int(os.environ.get("K_MID_JG", "2"))
PSP_BUFS = int(os.environ.get("K_PSP_BUFS", "2"))
GRP_BUFS = int(os.environ.get("K_GRP_BUFS", "8"))
HEADJ = int(os.environ.get("K_HEADJ", "4"))  # starter j-tiles (0 = off)
POOL_MODE = os.environ.get("K_POOL_MODE", "stack")

_CACHED_NC = None
_DVE_OPS = {}


def _register_dve_op(name, body_fn, ref):
    """Register a custom DVE op once; returns the DveOp."""
    if name in _DVE_OPS:
        return _DVE_OPS[name]
    import concourse.dve_ops as dvo
    from concourse.dve_spec import Spec, lower
    from concourse.dve_uop import DveOpSpec

    existing = [o for o in dvo.OPS if o.name == name]
    if existing:
        _DVE_OPS[name] = existing[0]
        return existing[0]

    op = dvo.DveOp(name, Spec(body=body_fn(), reference=ref), subdim=False,
                   uops_sha={})
    dvo.OPS.append(op)
    dvo.CUSTOM_DVE_SPECS[name] = op.spec
    dvo._SUB_OPCODE_FOR_NAME[name] = dvo._CUSTOM_DVE_ROW_BASE + len(dvo.OPS) - 1
    assert dvo._SUB_OPCODE_FOR_NAME[name] < 0x20
    shas = {}
    for ver in ("v3", "v4"):
        s = DveOpSpec(name=name, opcode=dvo.get_dve_sub_opcode(name),
                      uops=lower(op.spec, ver=ver), rd1_en=True)
        shas[ver] = s.sha(ver)
    object.__setattr__(op, "uops_sha", shas)
    _DVE_OPS[name] = op
    return op


def _register_add1p_recip():
    """Custom DVE op: out = recip_approx(in0 + in1 + 1), 1 Newton step."""
    import numpy as np_
    from concourse.dve_spec import AluOp, Bin, C0, C1, One, Src0, Src1

    def _body():
        _x = (Src0 + Src1) + One
        _nx = Bin(AluOp.BITWISE_NOT, _x, _x)
        _y0 = _nx * C0
        return _y0 * (C1 - _x * _y0)

    def _ref(in0, in1, s0, s1, imm2):
        x = (in0 + in1 + 1.0).astype(np_.float32)
        nx = (~x.view(np_.int32)).view(np_.float32)
        y0 = nx * np_.float32(s0)
        return y0 * (np_.float32(s1) - x * y0)

    return _register_dve_op("ADD1P_RECIP_1NR_ANT", _body, _ref)


def _build_nc():
    from concourse import bacc, tile
    from concourse.bass import mybir
    from concourse.dve_ops import RECIP_APPROX_FAST_CONSTS

    add1p_recip = _register_add1p_recip()

    f32 = mybir.dt.float32
    f16 = mybir.dt.float16
    bf16 = mybir.dt.bfloat16
    Exp = mybir.ActivationFunctionType.Exp
    rc = RECIP_APPROX_FAST_CONSTS

    nc = bacc.Bacc("TRN2", target_bir_lowering=False, debug=False)

    # kq[bi] = [kst_b | qst_b] for b = bi+1: kst rows 0:64 = k_b^T,
    # 64:128 = k_0^T; qst rows 0:64 = q_b^T, 64:128 = -q_0^T.
    kq_in = nc.dram_tensor("kq", [3, 2, 128, S], f16, kind="ExternalInput").ap()
    v_in = nc.dram_tensor("v", [B, 128, JT, D], bf16, kind="ExternalInput").ap()
    out_d = nc.dram_tensor("out", [B, D, S], f32, kind="ExternalOutput").ap()
    # starter blob: kst/qst slices for the first HEADJ j-tiles of chunk 0 in
    # one small leading DMA so the pipeline ramps before the full kq loads:
    # [kst1h | kst2h | kst3h | qst1c0 | qst2c0 | qst3c0]
    W_HEAD = 3 * HEADJ * 128 + 3 * ICW
    qkh_in = (nc.dram_tensor("qkh", [128, W_HEAD], f16,
                             kind="ExternalInput").ap() if HEADJ else None)

    with tile.TileContext(nc, pool_alloc_mode=POOL_MODE) as tc:
        with (
            tc.tile_pool(name="wts", bufs=1) as wpool,
            tc.tile_pool(name="grp", bufs=GRP_BUFS) as gpool,
            tc.tile_pool(name="osb", bufs=2) as opool,
            tc.tile_pool(name="ps", bufs=PSP_BUFS, space="PSUM") as psp,
            tc.tile_pool(name="po", bufs=1, space="PSUM") as pop,
        ):
            KQ = [wpool.tile([128, 2, S], f16, tag=f"kq{b}", name=f"KQ{b}")
                  for b in range(3)]
            V = wpool.tile([128, B, JT, D], bf16, tag="v")
            if HEADJ:
                ST = wpool.tile([128, W_HEAD], f16, tag="qkh")
                nc.sync.dma_start(out=ST[:], in_=qkh_in)
            for b in range(3):
                nc.sync.dma_start(out=KQ[b][:],
                                  in_=kq_in[b].transpose([1, 0, 2]))
            nc.sync.dma_start(out=V[:], in_=v_in.transpose([1, 0, 2, 3]))

            def schedule(ic):
                # taper group size down at every chunk boundary: big WV
                # bunches queued at a boundary block the next chunk's diff
                # matmuls in the PE FIFO and stall VectorE 2-4us per chunk
                if MID_JG == 2:
                    if ic == 0:
                        return [1, 1] + [2] * 7
                    if ic == IC - 1:
                        return [2] * 7 + [1, 1]
                    return [2] * 8
                if ic == 0:
                    return [1, 1, 2, 4, 4, 2, 2]
                if ic == IC - 1:
                    return [2, 4, 4, 2, 2, 1, 1]
                return [2, 4, 4, 4, 2]

            def emit_group(ic, JG, j0, out_t):
                isl = slice(ic * ICW, (ic + 1) * ICW)
                E4 = gpool.tile([128, JG, 3, ICW], bf16, tag="E4")
                for j4 in range(JG):
                    j = j0 + j4
                    sp = psp.tile([128, 3, ICW], f32, tag="sp")
                    for b in range(3):
                        if HEADJ and ic == 0 and j < HEADJ:
                            kt = ST[:, (b * HEADJ + j) * 128:
                                    (b * HEADJ + j + 1) * 128]
                            qt = ST[:, 3 * HEADJ * 128 + b * ICW:
                                    3 * HEADJ * 128 + (b + 1) * ICW]
                        else:
                            kt = KQ[b][:, 0, j * 128:(j + 1) * 128]
                            qt = KQ[b][:, 1, isl]
                        # d_b_T[j, i] = k_b.q_b - k_0.q_0 (K=128 stacked)
                        nc.tensor.matmul(sp[:, b, :], kt, qt,
                                         start=True, stop=True)
                    nc.scalar.activation(E4[:, j4, :, :], sp[:], Exp,
                                         scale=SCALE)

                # SA = e1 + e2; R = 1 / (1 + SA + e3) = w0.  Pair-granular so
                # VectorE can start as soon as two exps land, not the whole
                # group.
                SA = gpool.tile([128, JG, ICW], bf16, tag="SA")
                R = gpool.tile([128, JG, ICW], bf16, tag="R")
                for p in range(0, JG, 2):
                    pe = min(p + 2, JG)
                    nc.vector.tensor_add(SA[:, p:pe, :], E4[:, p:pe, 0, :],
                                         E4[:, p:pe, 1, :])
                    nc.vector._custom_dve(
                        add1p_recip, out=R[:, p:pe, :], in0=SA[:, p:pe, :],
                        in1=E4[:, p:pe, 2, :], s0=rc["s0"], s1=rc["s1"])
                # W[p, j4, b-1, i] = e_b * r for b in 1..3 (fp16 out: the
                # bf16-in/fp16-out TT combo measures formula-exact 2x)
                W = gpool.tile([128, JG, 3, ICW], f16, tag="W")
                rb = R[:].unsqueeze(2).broadcast_to([128, JG, 3, ICW])
                nc.vector.tensor_mul(W[:], E4[:], rb)

                # out_T[v, i] += col-tiled accumulation over j
                for j4 in range(JG):
                    j = j0 + j4
                    rhs = [R[:, j4, :], W[:, j4, 0, :],
                           W[:, j4, 1, :], W[:, j4, 2, :]]
                    for b, (pair, base) in enumerate(
                        [(0, 0), (0, 64), (1, 0), (1, 64)]
                    ):
                        nc.tensor.matmul(
                            out_t[base:base + 64, pair, :],
                            V[:, b, j, :], rhs[b],
                            start=(j == 0), stop=(j == JT - 1),
                            tile_position=(0, base), skip_group_check=True)

            def emit_outputs(ic, out_t):
                isl = slice(ic * ICW, (ic + 1) * ICW)
                OSB = opool.tile([128, 2, ICW], f32, tag="osb")
                last = ic == IC - 1
                eng23 = nc.scalar if last else nc.sync
                if last:
                    # VectorE is idle in the drain chunk: split the copy
                    nc.scalar.copy(OSB[:, 0, :], out_t[:, 0, :])
                    nc.vector.tensor_copy(OSB[:, 1, :], out_t[:, 1, :])
                else:
                    nc.scalar.copy(OSB[:], out_t[:])
                nc.sync.dma_start(out=out_d[0, :, isl], in_=OSB[0:64, 0, :])
                nc.sync.dma_start(out=out_d[1, :, isl], in_=OSB[64:128, 0, :])
                eng23.dma_start(out=out_d[2, :, isl], in_=OSB[0:64, 1, :])
                eng23.dma_start(out=out_d[3, :, isl], in_=OSB[64:128, 1, :])

            for ic in range(IC):
                out_t = pop.tile([128, 2, ICW], f32, tag="oacc")
                j0 = 0
                for JG in schedule(ic):
                    emit_group(ic, JG, j0, out_t)
                    j0 += JG
                emit_outputs(ic, out_t)

    nc.compile()
    return nc


def _get_nc():
    global _CACHED_NC
    if _CACHED_NC is None:
        _CACHED_NC = _build_nc()
    return _CACHED_NC


def _make_in_maps(query, key, value):
    import ml_dtypes
    q16 = query.astype(np.float16)
    k16 = key.astype(np.float16)
    vbf = value.astype(ml_dtypes.bfloat16)
    in_maps = []
    for h in range(H):
        qt = q16[:, h].transpose(0, 2, 1)  # [B, D, S]
        kt = k16[:, h].transpose(0, 2, 1)
        nq0 = -qt[0]
        kq = np.empty((3, 2, 128, S), np.float16)
        for bi in range(3):
            kq[bi, 0, 0:64] = kt[bi + 1]
            kq[bi, 0, 64:128] = kt[0]
            kq[bi, 1, 0:64] = qt[bi + 1]
            kq[bi, 1, 64:128] = nq0
        vv = vbf[:, h].reshape(B, JT, 128, D).transpose(0, 2, 1, 3)
        im = {
            "kq": np.ascontiguousarray(kq),
            "v": np.ascontiguousarray(vv),
        }
        if HEADJ:
            im["qkh"] = np.ascontiguousarray(np.concatenate(
                [kq[bi, 0, :, 0:HEADJ * 128] for bi in range(3)]
                + [kq[bi, 1, :, 0:ICW] for bi in range(3)], axis=1))
        in_maps.append(im)
    return in_maps


def _assemble(results):
    out = np.empty((B, H, S, D), np.float32)
    for h in range(H):
        out[:, h] = results[h]["out"].transpose(0, 2, 1)  # [B,D,S] -> [B,S,D]
    return out


def _install_profile_hook():
    """Provide antenv.axon_hooks with a ctypes NTFF profile hook."""
    import contextlib
    import ctypes
    import types

    try:
        from antenv.axon_hooks import get_axon_ntff_profile_hook  # noqa: F401
        return
    except ImportError:
        pass

    so_path = "/opt/axon/libaxon_pjrt.so"
    lib = ctypes.CDLL(so_path)
    if not hasattr(lib, "axon_start_nrt_profile"):
        return
    lib.axon_start_nrt_profile.argtypes = [
        ctypes.POINTER(ctypes.c_int64), ctypes.c_size_t]
    lib.axon_start_nrt_profile.restype = ctypes.c_int64
    lib.axon_stop_nrt_profile.argtypes = [ctypes.c_char_p]
    lib.axon_stop_nrt_profile.restype = ctypes.c_int64

    @contextlib.contextmanager
    def _hook(output_dir, device_ids):
        import jax
        jax.devices()
        if device_ids:
            ids = (ctypes.c_int64 * len(device_ids))(*device_ids)
            rc = lib.axon_start_nrt_profile(ids, len(device_ids))
        else:
            rc = lib.axon_start_nrt_profile(None, 0)
        if rc != 0:
            raise RuntimeError(f"axon_start_nrt_profile rc={rc}")
        try:
            yield
        finally:
            n = lib.axon_stop_nrt_profile(str(output_dir).encode())
            print(f"ntff profile: {n} file(s) written to {output_dir}")

    mod = types.ModuleType("antenv.axon_hooks")
    mod.get_axon_ntff_profile_hook = lambda: _hook
    mod.set_axon_ntff_profile_hook = lambda h: None
    sys.modules["antenv.axon_hooks"] = mod


def run(query, key, value, trace=False):
    """Run the distributed kernel; returns (output, exec_time_ns or None)."""
    from concourse.bass_utils import run_bass_kernel_spmd

    if trace:
        _install_profile_hook()
    nc = _get_nc()
    in_maps = _make_in_maps(query, key, value)
    res = run_bass_kernel_spmd(nc, in_maps, core_ids=list(range(N_CORES)),
                               trace=trace)
    return _assemble(res.results), res.exec_time_ns


def kernel(query, key, value):
    out, _ = run(query, key, value, trace=False)
    return out


# revision 16
# speedup vs baseline: 1.0148x; 1.0048x over previous
"""Batch-softmax attention for Trainium2 (8 NeuronCores) — diff-softmax.

Problem: out[b,h,i,v] = sum_j softmax_over_b(QK^T/sqrt(H))[b,h,i,j] * V[b,h,j,v]
with B=4, H=8, S=2048, D=64.  Softmax over the BATCH axis (dim=0).
Sharding: one head per NeuronCore (H=8), so the batch softmax is core-local.

Restructure vs the previous 170.7us kernel (which was ScalarE+VectorE
co-bound: exp ACTIVATE 150.8us busy, DVE 163us busy):

  softmax_b(s_0..s_3) = [r, e1*r, e2*r, e3*r],  e_b = exp(s_b - s_0),
                        r = 1/(1 + e1 + e2 + e3)

  - PE computes the DIFFS d_b = (q_b.k_b - q_0.k_0)/1 directly as ONE K=128
    matmul per b in {1,2,3}: lhsT = [k_b^T; k_0^T] (64+64 rows stacked),
    rhs = [q_b^T; -q_0^T].  3 score streams instead of 4.
  - ScalarE: ONE exp per j-tile over [128, 3*512] PSUM -> E bf16
    (4 -> 3 exp units: ACTIVATE 150.8 -> 102.5us busy).
  - VectorE: SA = e1+e2 (TT 2x), R = recip_1NR(1 + SA + e3) (custom DVE op
    with the +1 folded in via the One const lane; R = w0 directly, so b=0
    needs no multiply), W123 = E * bcast(R) fp16 (TT 2x).
    DVE 163 -> 119us busy.
  - WV matmuls: V bf16 stationary, rhs = R (bf16) / W123 (fp16) planes,
    col-tiled pairs (M=64 at (0,0)/(0,64)) accumulating out_T[v,i] in one
    PSUM tile.

PSUM: 2 rolling score tiles x 3 banks + 1 accumulator tile x 2 banks = 8.
Measured on trn2 (8 cores): 150.1-152.2us NEFF exec across repeats
(run-to-run noise +-1.5us), rel L2 err 2.3e-3 (baseline it replaces:
171.2us / 2.1e-3).

Engine accounting at 150us: VectorE 118us busy (92% packed in its window)
is the pacer; ScalarE 102.5us; PE ~75% busy mostly HAM-warm; ~14us
preamble+ramp, ~8us drain/postamble.  Rejected variants (measured): SA on
GPSIMD 198us (Pool sem/dispatch overhead + SBUF-port contention degrades
concurrent DVE 2x ops to 1x); e1+e2 via PE identity-matmul accumulation
184us (PE FIFO head-of-line blocking on the exp->idMM dependency).
"""

import math
import os
import sys

import numpy as np

sys.path.insert(0, "/opt/trn_rl_repo")
os.environ.setdefault("MYCRO_LOCAL_CACHE", "1")

B, H, S, D = 4, 8, 2048, 64
N_CORES = 8
SCALE = 1.0 / math.sqrt(H)  # NOTE: reference scales by sqrt(num_heads)

IC = 4          # i-chunks of 512 columns
ICW = S // IC   # 512
JT = S // 128   # 16 j-tiles of 128 rows
MID_JG = int(os.environ.get("K_MID_JG", "2"))
PSP_BUFS = int(os.environ.get("K_PSP_BUFS", "2"))
GRP_BUFS = int(os.environ.get("K_GRP_BUFS", "8"))
HEADJ = int(os.environ.get("K_HEADJ", "4"))  # starter j-tiles (0 = off)
POOL_MODE = os.environ.get("K_POOL_MODE", "stack")

_CACHED_NC = None
_DVE_OPS = {}


def _register_dve_op(name, body_fn, ref):
    """Register a custom DVE op once; returns the DveOp."""
    if name in _DVE_OPS:
        return _DVE_OPS[name]
    import concourse.dve_ops as dvo
    from concourse.dve_spec import Spec, lower
    from concourse.dve_uop import DveOpSpec

    existing = [o for o in dvo.OPS if o.name == name]
    if existing:
        _DVE_OPS[name] = existing[0]
        return existing[0]

    op = dvo.DveOp(name, Spec(body=body_fn(), reference=ref), subdim=False,
                   uops_sha={})
    dvo.OPS.append(op)
    dvo.CUSTOM_DVE_SPECS[name] = op.spec
    dvo._SUB_OPCODE_FOR_NAME[name] = dvo._CUSTOM_DVE_ROW_BASE + len(dvo.OPS) - 1
    assert dvo._SUB_OPCODE_FOR_NAME[name] < 0x20
    shas = {}
    for ver in ("v3", "v4"):
        s = DveOpSpec(name=name, opcode=dvo.get_dve_sub_opcode(name),
                      uops=lower(op.spec, ver=ver), rd1_en=True)
        shas[ver] = s.sha(ver)
    object.__setattr__(op, "uops_sha", shas)
    _DVE_OPS[name] = op
    return op


def _register_add1p_recip():
    """Custom DVE op: out = recip_approx(in0 + in1 + 1), 1 Newton step."""
    import numpy as np_
    from concourse.dve_spec import AluOp, Bin, C0, C1, One, Src0, Src1

    def _body():
        _x = (Src0 + Src1) + One
        _nx = Bin(AluOp.BITWISE_NOT, _x, _x)
        _y0 = _nx * C0
        return _y0 * (C1 - _x * _y0)

    def _ref(in0, in1, s0, s1, imm2):
        x = (in0 + in1 + 1.0).astype(np_.float32)
        nx = (~x.view(np_.int32)).view(np_.float32)
        y0 = nx * np_.float32(s0)
        return y0 * (np_.float32(s1) - x * y0)

    return _register_dve_op("ADD1P_RECIP_1NR_ANT", _body, _ref)


def _build_nc():
    from concourse import bacc, tile
    from concourse.bass import mybir
    from concourse.dve_ops import RECIP_APPROX_FAST_CONSTS

    add1p_recip = _register_add1p_recip()

    f32 = mybir.dt.float32
    f16 = mybir.dt.float16
    bf16 = mybir.dt.bfloat16
    Exp = mybir.ActivationFunctionType.Exp
    rc = RECIP_APPROX_FAST_CONSTS

    nc = bacc.Bacc("TRN2", target_bir_lowering=False, debug=False)

    # kq[bi] = [kst_b | qst_b] for b = bi+1: kst rows 0:64 = k_b^T,
    # 64:128 = k_0^T; qst rows 0:64 = q_b^T, 64:128 = -q_0^T.
    kq_in = nc.dram_tensor("kq", [3, 2, 128, S], f16, kind="ExternalInput").ap()
    v_in = nc.dram_tensor("v", [B, 128, JT, D], bf16, kind="ExternalInput").ap()
    out_d = nc.dram_tensor("out", [B, D, S], f32, kind="ExternalOutput").ap()
    # starter blob: kst/qst slices for the first HEADJ j-tiles of chunk 0 in
    # one small leading DMA so the pipeline ramps before the full kq loads:
    # [kst1h | kst2h | kst3h | qst1c0 | qst2c0 | qst3c0]
    W_HEAD = 3 * HEADJ * 128 + 3 * ICW
    qkh_in = (nc.dram_tensor("qkh", [128, W_HEAD], f16,
                             kind="ExternalInput").ap() if HEADJ else None)

    with tile.TileContext(nc, pool_alloc_mode=POOL_MODE) as tc:
        with (
            tc.tile_pool(name="wts", bufs=1) as wpool,
            tc.tile_pool(name="grp", bufs=GRP_BUFS) as gpool,
            tc.tile_pool(name="osb", bufs=2) as opool,
            tc.tile_pool(name="ps", bufs=PSP_BUFS, space="PSUM") as psp,
            tc.tile_pool(name="po", bufs=1, space="PSUM") as pop,
        ):
            KQ = [wpool.tile([128, 2, S], f16, tag=f"kq{b}", name=f"KQ{b}")
                  for b in range(3)]
            V = wpool.tile([128, B, JT, D], bf16, tag="v")
            if HEADJ:
                ST = wpool.tile([128, W_HEAD], f16, tag="qkh")
                nc.sync.dma_start(out=ST[:], in_=qkh_in)
            for b in range(3):
                nc.sync.dma_start(out=KQ[b][:],
                                  in_=kq_in[b].transpose([1, 0, 2]))
            nc.sync.dma_start(out=V[:], in_=v_in.transpose([1, 0, 2, 3]))

            def schedule(ic):
                # taper group size down at every chunk boundary: big WV
                # bunches queued at a boundary block the next chunk's diff
                # matmuls in the PE FIFO and stall VectorE 2-4us per chunk
                if MID_JG == 2:
                    if ic == 0:
                        return [1, 1] + [2] * 7
                    if ic == IC - 1:
                        return [2] * 7 + [1, 1]
                    return [2] * 8
                if ic == 0:
                    return [1, 1, 2, 4, 4, 2, 2]
                if ic == IC - 1:
                    return [2, 4, 4, 2, 2, 1, 1]
                return [2, 4, 4, 4, 2]

            def emit_group(ic, JG, j0, out_t):
                isl = slice(ic * ICW, (ic + 1) * ICW)
                E4 = gpool.tile([128, JG, 3, ICW], bf16, tag="E4")
                for j4 in range(JG):
                    j = j0 + j4
                    sp = psp.tile([128, 3, ICW], f32, tag="sp")
                    for b in range(3):
                        if HEADJ and ic == 0 and j < HEADJ:
                            kt = ST[:, (b * HEADJ + j) * 128:
                                    (b * HEADJ + j + 1) * 128]
                            qt = ST[:, 3 * HEADJ * 128 + b * ICW:
                                    3 * HEADJ * 128 + (b + 1) * ICW]
                        else:
                            kt = KQ[b][:, 0, j * 128:(j + 1) * 128]
                            qt = KQ[b][:, 1, isl]
                        # d_b_T[j, i] = k_b.q_b - k_0.q_0 (K=128 stacked)
                        nc.tensor.matmul(sp[:, b, :], kt, qt,
                                         start=True, stop=True)
                    nc.scalar.activation(E4[:, j4, :, :], sp[:], Exp,
                                         scale=SCALE)

                # SA = e1 + e2; R = 1 / (1 + SA + e3) = w0.  Pair-granular so
                # VectorE can start as soon as two exps land, not the whole
                # group.
                SA = gpool.tile([128, JG, ICW], bf16, tag="SA")
                R = gpool.tile([128, JG, ICW], bf16, tag="R")
                for p in range(0, JG, 2):
                    pe = min(p + 2, JG)
                    nc.vector.tensor_add(SA[:, p:pe, :], E4[:, p:pe, 0, :],
                                         E4[:, p:pe, 1, :])
                    nc.vector._custom_dve(
                        add1p_recip, out=R[:, p:pe, :], in0=SA[:, p:pe, :],
                        in1=E4[:, p:pe, 2, :], s0=rc["s0"], s1=rc["s1"])
                # W[p, j4, b-1, i] = e_b * r for b in 1..3 (fp16 out: the
                # bf16-in/fp16-out TT combo measures formula-exact 2x)
                W = gpool.tile([128, JG, 3, ICW], f16, tag="W")
                rb = R[:].unsqueeze(2).broadcast_to([128, JG, 3, ICW])
                nc.vector.tensor_mul(W[:], E4[:], rb)

                # out_T[v, i] += col-tiled accumulation over j
                for j4 in range(JG):
                    j = j0 + j4
                    rhs = [R[:, j4, :], W[:, j4, 0, :],
                           W[:, j4, 1, :], W[:, j4, 2, :]]
                    for b, (pair, base) in enumerate(
                        [(0, 0), (0, 64), (1, 0), (1, 64)]
                    ):
                        nc.tensor.matmul(
                            out_t[base:base + 64, pair, :],
                            V[:, b, j, :], rhs[b],
                            start=(j == 0), stop=(j == JT - 1),
                            tile_position=(0, base), skip_group_check=True)

            def emit_outputs(ic, out_t):
                isl = slice(ic * ICW, (ic + 1) * ICW)
                OSB = opool.tile([128, 2, ICW], f32, tag="osb")
                last = ic == IC - 1
                eng23 = nc.scalar if last else nc.sync
                if last:
                    # VectorE is idle in the drain chunk: split the copy
                    nc.scalar.copy(OSB[:, 0, :], out_t[:, 0, :])
                    nc.vector.tensor_copy(OSB[:, 1, :], out_t[:, 1, :])
                else:
                    nc.scalar.copy(OSB[:], out_t[:])
                nc.sync.dma_start(out=out_d[0, :, isl], in_=OSB[0:64, 0, :])
                nc.sync.dma_start(out=out_d[1, :, isl], in_=OSB[64:128, 0, :])
                eng23.dma_start(out=out_d[2, :, isl], in_=OSB[0:64, 1, :])
                eng23.dma_start(out=out_d[3, :, isl], in_=OSB[64:128, 1, :])

            for ic in range(IC):
                out_t = pop.tile([128, 2, ICW], f32, tag="oacc")
                j0 = 0
                for JG in schedule(ic):
                    emit_group(ic, JG, j0, out_t)
                    j0 += JG
                emit_outputs(ic, out_t)

    nc.compile()
    return nc


def _get_nc():
    global _CACHED_NC
    if _CACHED_NC is None:
        _CACHED_NC = _build_nc()
    return _CACHED_NC


def _make_in_maps(query, key, value):
    import ml_dtypes
    q16 = query.astype(np.float16)
    k16 = key.astype(np.float16)
    vbf = value.astype(ml_dtypes.bfloat16)
    in_maps = []
    for h in range(H):
        qt = q16[:, h].transpose(0, 2, 1)  # [B, D, S]
        kt = k16[:, h].transpose(0, 2, 1)
        nq0 = -qt[0]
        kq = np.empty((3, 2, 128, S), np.float16)
        for bi in range(3):
            kq[bi, 0, 0:64] = kt[bi + 1]
            kq[bi, 0, 64:128] = kt[0]
            kq[bi, 1, 0:64] = qt[bi + 1]
            kq[bi, 1, 64:128] = nq0
        vv = vbf[:, h].reshape(B, JT, 128, D).transpose(0, 2, 1, 3)
        im = {
            "kq": np.ascontiguousarray(kq),
            "v": np.ascontiguousarray(vv),
        }
        if HEADJ:
            im["qkh"] = np.ascontiguousarray(np.concatenate(
                [kq[bi, 0, :, 0:HEADJ * 128] for bi in range(3)]
                + [kq[bi, 1, :, 0:ICW] for bi in range(3)], axis=1))
        in_maps.append(im)
    return in_maps


def _assemble(results):
    out = np.empty((B, H, S, D), np.float32)
    for h in range(H):
        out[:, h] = results[h]["out"].transpose(0, 2, 1)  # [B,D,S] -> [B,S,D]
    return out


def _install_profile_hook():
    """Provide antenv.axon_hooks with a ctypes NTFF profile hook."""
    import contextlib
    import ctypes
    import types

    try:
        from antenv.axon_hooks import get_axon_ntff_profile_hook  # noqa: F401
        return
    except ImportError:
        pass

    so_path = "/opt/axon/libaxon_pjrt.so"
    lib = ctypes.CDLL(so_path)
    if not hasattr(lib, "axon_start_nrt_profile"):
        return
    lib.axon_start_nrt_profile.argtypes = [
        ctypes.POINTER(ctypes.c_int64), ctypes.c_size_t]
    lib.axon_start_nrt_profile.restype = ctypes.c_int64
    lib.axon_stop_nrt_profile.argtypes = [ctypes.c_char_p]
    lib.axon_stop_nrt_profile.restype = ctypes.c_int64

    @contextlib.contextmanager
    def _hook(output_dir, device_ids):
        import jax
        jax.devices()
        if device_ids:
            ids = (ctypes.c_int64 * len(device_ids))(*device_ids)
            rc = lib.axon_start_nrt_profile(ids, len(device_ids))
        else:
            rc = lib.axon_start_nrt_profile(None, 0)
        if rc != 0:
            raise RuntimeError(f"axon_start_nrt_profile rc={rc}")
        try:
            yield
        finally:
            n = lib.axon_stop_nrt_profile(str(output_dir).encode())
            print(f"ntff profile: {n} file(s) written to {output_dir}")

    mod = types.ModuleType("antenv.axon_hooks")
    mod.get_axon_ntff_profile_hook = lambda: _hook
    mod.set_axon_ntff_profile_hook = lambda h: None
    sys.modules["antenv.axon_hooks"] = mod


def run(query, key, value, trace=False):
    """Run the distributed kernel; returns (output, exec_time_ns or None)."""
    from concourse.bass_utils import run_bass_kernel_spmd

    if trace:
        _install_profile_hook()
    nc = _get_nc()
    in_maps = _make_in_maps(query, key, value)
    res = run_bass_kernel_spmd(nc, in_maps, core_ids=list(range(N_CORES)),
                               trace=trace)
    return _assemble(res.results), res.exec_time_ns


def kernel(query, key, value):
    out, _ = run(query, key, value, trace=False)
    return out
